# revision 1
# baseline (speedup 1.0000x reference)
# Trainium2 Bass kernel for nn_CovariantPotentialNet (B=4096, D=64, K=64, DM=512).
#
# The network collapses algebraically: tokens_x[b] = diag(rw[b]) @ chart_emb is
# rank-structured, so every DM=512-wide projection folds into small per-chart
# constants computed once on the host:
#   scores[b,k] = rw[b,k] * (z[b] @ A + a0)[k] / sqrt(DM) - geo * acosh(arg)^2
#   arg[b,k]    = 1 + 2*diff2[b,k] / ((1-|z[b]|^2) * (1-|c_k|^2))
#   out[b]      = sum_k softmax(scores)[b,k] * rw[b,k] * e[k] + e0
# with A [D,K], a0 [K], e [K], e0 scalar folded from the weight matrices
# (spectral norms included). The device kernel is pure data parallel over B:
# each of the 8 cores processes 512 rows.
import numpy as np
import sys

for _p in ('/opt/trn_rl_repo', '/root/.axon_site/_ro/trn_rl_repo'):
    if _p not in sys.path:
        sys.path.append(_p)

import concourse.bass as bass
import concourse.mybir as mybir
import concourse.tile as tile
import concourse.bacc as bacc
from concourse.bass_utils import run_bass_kernel_spmd

F32 = mybir.dt.float32
N_CORES = 8
B, D, K, DM = 4096, 64, 64, 512
BC = B // N_CORES          # 512 rows per core
NT = BC // 128             # 4 tiles of 128 rows
ALU = mybir.AluOpType
ACTF = mybir.ActivationFunctionType


def _fold_constants(inputs):
    """Host-side folding of all weights into small per-chart constants (float64)."""
    ii = {k: np.asarray(v).astype(np.float64) for k, v in inputs.items()}

    def l2n(x):
        return x / (np.linalg.norm(x) + 1e-12)

    def sscale(W, iters=5):
        u = l2n(np.ones(W.shape[0]))
        v = l2n(W.T @ u)
        for _ in range(iters):
            v = l2n(W.T @ u)
            u = l2n(W @ v)
        return W / (u @ (W @ v))

    Wz = sscale(ii['zW'])                     # [DM, D]
    vWs = sscale(ii['vW'])                    # [1, DM]
    cc = ii['chart_centers']
    n = np.linalg.norm(cc, axis=-1, keepdims=True)
    ccp = cc * np.minimum(1.0, (1.0 - 1e-5) / np.maximum(n, 1e-12))   # [K, D]
    cn = np.sum(ccp * ccp, axis=-1)           # [K]
    cdiv = 1.0 - cn                           # [K] (positive: centers inside ball)

    Ek = ii['chart_emb'] @ ii['Wk'].T         # [K, DM]
    Ev = ii['chart_emb'] @ ii['Wv'].T         # [K, DM]
    A = Wz.T @ (ii['Wq'].T @ Ek.T)            # [D, K]
    a0 = (ii['zb'] @ ii['Wq'].T + ii['bq']) @ Ek.T     # [K]
    h = ii['Wo'].T @ vWs[0]                   # [DM]
    e = Ev @ h                                # [K]
    e0 = float(ii['bv'] @ h + ii['bo'] @ vWs[0] + ii['vb'][0])
    geo = float(ii['geo_scale'])

    # Matmul weights, contraction dim on partitions (d = 0..63):
    #   G output col k       (0..63):  S1 part   = z @ A          (+ a0 via CONSTA)
    #   G output col 64+k:   geo part  = (zn - 2 z.c + cn)/cdiv   (+ cn/cdiv via CONSTA)
    #   col 128: zn column (coef 1 on z^2 rows only)
    Gz = np.zeros((D, 129))
    Gz[:, 0:K] = A
    Gz[:, K:2 * K] = (-2.0 * ccp / cdiv[:, None]).T
    Gs = np.zeros((D, 129))
    Gs[:, K:2 * K] = np.broadcast_to((1.0 / cdiv)[None, :], (D, K))
    Gs[:, 128] = 1.0

    crow = np.zeros(128)
    crow[0:K] = a0
    crow[K:2 * K] = cn / cdiv

    consts = {
        'Gz': Gz.astype(np.float32),
        'Gs': Gs.astype(np.float32),
        'CONSTA': np.ascontiguousarray(
            np.broadcast_to(crow.astype(np.float32), (128, NT, 128))),
        'CONSTB': np.ascontiguousarray(
            np.broadcast_to(e.astype(np.float32), (128, NT, K))),
        'IDENT': np.eye(128, dtype=np.float32),
        'geo': np.float32(geo),
        'e0': e0,
        'inv_sqrt': np.float32(1.0 / np.sqrt(float(DM))),
    }
    return consts


def _build_program(consts):
    nc = bacc.Bacc()
    z_in = nc.dram_tensor("z_in", [BC, D], F32, kind="ExternalInput")
    rw_in = nc.dram_tensor("rw_in", [BC, K], F32, kind="ExternalInput")
    res_out = nc.dram_tensor("res_out", [128, NT], F32, kind="ExternalOutput")

    gz_d = nc.inline_tensor(consts['Gz'], name="c_gz")
    gs_d = nc.inline_tensor(consts['Gs'], name="c_gs")
    ca_d = nc.inline_tensor(consts['CONSTA'], name="c_ca")
    cb_d = nc.inline_tensor(consts['CONSTB'], name="c_cb")
    id_d = nc.inline_tensor(consts['IDENT'], name="c_id")

    geo = float(consts['geo'])
    inv_sqrt = float(consts['inv_sqrt'])

    with tile.TileContext(nc) as tc:
        with (
            tc.tile_pool(name="sb", bufs=1) as sb,
            tc.tile_pool(name="ps", bufs=1, space=bass.MemorySpace.PSUM) as ps,
            tc.tile_pool(name="pst", bufs=2, space=bass.MemorySpace.PSUM) as pst,
        ):
            z_all = sb.tile([128, NT, D], F32)
            rw_all = sb.tile([128, NT, K], F32)
            ident = sb.tile([128, 128], F32)
            gz = sb.tile([D, 129], F32)
            gs = sb.tile([D, 129], F32)
            consta = sb.tile([128, NT, 128], F32)
            constb = sb.tile([128, NT, K], F32)

            nc.sync.dma_start(z_all[:], z_in.rearrange("(t p) d -> p t d", p=128))
            nc.sync.dma_start(rw_all[:], rw_in.rearrange("(t p) d -> p t d", p=128))
            nc.sync.dma_start(ident[:], id_d[:])
            nc.sync.dma_start(gz[:], gz_d[:])
            nc.sync.dma_start(gs[:], gs_d[:])
            nc.sync.dma_start(consta[:], ca_d[:])
            nc.sync.dma_start(constb[:], cb_d[:])

            # negative-one bias column for the Sqrt activation
            neg1 = sb.tile([128, 1], F32)
            nc.vector.memset(neg1[:], -1.0)

            zzt = sb.tile([D, NT, 128], F32)    # z.T  per tile
            zsq = sb.tile([D, NT, 128], F32)    # (z.T)^2 per tile
            psum_g = ps.tile([128, NT, 128], F32)
            psum_zn = ps.tile([128, NT], F32)

            for t in range(NT):
                tp = pst.tile([D, 128], F32)
                nc.tensor.transpose(tp[:], z_all[:, t, :], ident[:])
                nc.scalar.copy(zzt[:, t, :], tp[:])
                nc.vector.tensor_mul(zsq[:, t, :], zzt[:, t, :], zzt[:, t, :])
                # main columns: accumulate z-part and z^2-part
                nc.tensor.matmul(psum_g[:, t, :], zzt[:, t, :], gz[:, 0:128],
                                 start=True, stop=False)
                nc.tensor.matmul(psum_g[:, t, :], zsq[:, t, :], gs[:, 0:128],
                                 start=False, stop=True)
                # zn column (z^2 rows only)
                nc.tensor.matmul(psum_zn[:, t:t + 1], zsq[:, t, :], gs[:, 128:129],
                                 start=True, stop=True)

            # G = psum + CONSTA   (adds a0 to S1 cols, cn/cdiv to geo cols)
            g = sb.tile([128, NT, 128], F32)
            nc.vector.tensor_add(g[:], psum_g[:], consta[:])

            # izd = 2 / (1 - zn)
            zd = sb.tile([128, NT], F32)
            nc.vector.tensor_scalar(out=zd[:], in0=psum_zn[:], scalar1=-0.5,
                                    scalar2=0.5, op0=ALU.mult, op1=ALU.add)
            izd = sb.tile([128, NT], F32)
            nc.vector.reciprocal(izd[:], zd[:])

            # arg = max(1 + t2 * izd, 1 + 1e-7)
            arg = sb.tile([128, NT, K], F32)
            for t in range(NT):
                nc.vector.tensor_scalar(out=arg[:, t, :], in0=g[:, t, K:128],
                                        scalar1=izd[:, t:t + 1], scalar2=1.0,
                                        op0=ALU.mult, op1=ALU.add)
            nc.vector.tensor_scalar_max(arg[:], arg[:], 1.0 + 1e-7)

            # d2 = acosh(arg)^2 = ln(arg + sqrt(arg^2 - 1))^2
            u = sb.tile([128, NT, K], F32)
            nc.vector.tensor_mul(u[:], arg[:], arg[:])
            w = sb.tile([128, NT, K], F32)
            nc.scalar.activation(w[:], u[:], ACTF.Sqrt, bias=neg1[:], scale=1.0)
            t4 = sb.tile([128, NT, K], F32)
            nc.vector.tensor_add(t4[:], arg[:], w[:])
            dl = sb.tile([128, NT, K], F32)
            nc.scalar.activation(dl[:], t4[:], ACTF.Ln)
            dsq = sb.tile([128, NT, K], F32)
            nc.vector.tensor_mul(dsq[:], dl[:], dl[:])

            # scores = (S1c * inv_sqrt) * rw - geo * d2
            sc = sb.tile([128, NT, K], F32)
            nc.vector.scalar_tensor_tensor(out=sc[:], in0=g[:, :, 0:K],
                                           scalar=inv_sqrt, in1=rw_all[:],
                                           op0=ALU.mult, op1=ALU.mult)
            sco = sb.tile([128, NT, K], F32)
            nc.vector.scalar_tensor_tensor(out=sco[:], in0=dsq[:], scalar=-geo,
                                           in1=sc[:], op0=ALU.mult, op1=ALU.add)

            # softmax-weighted sum (scores are in [-2.3, -0.4]: no max-shift needed)
            p = sb.tile([128, NT, K], F32)
            nc.scalar.activation(p[:], sco[:], ACTF.Exp)
            s = sb.tile([128, NT], F32)
            nc.vector.reduce_sum(s[:], p[:], axis=mybir.AxisListType.X)
            rwe = sb.tile([128, NT, K], F32)
            nc.vector.tensor_mul(rwe[:], rw_all[:], constb[:])
            prw = sb.tile([128, NT, K], F32)
            nc.vector.tensor_mul(prw[:], p[:], rwe[:])
            num = sb.tile([128, NT], F32)
            nc.vector.reduce_sum(num[:], prw[:], axis=mybir.AxisListType.X)

            rs = sb.tile([128, NT], F32)
            nc.vector.reciprocal(rs[:], s[:])
            res = sb.tile([128, NT], F32)
            nc.vector.tensor_mul(res[:], num[:], rs[:])

            nc.sync.dma_start(res_out[:], res[:])

    nc.compile()
    return nc


def kernel(**inputs):
    consts = _fold_constants(inputs)
    nc = _build_program(consts)

    z = np.ascontiguousarray(np.asarray(inputs['z'], dtype=np.float32))
    rw = np.ascontiguousarray(np.asarray(inputs['rw'], dtype=np.float32))
    in_maps = [
        {
            "z_in": np.ascontiguousarray(z[c * BC:(c + 1) * BC]),
            "rw_in": np.ascontiguousarray(rw[c * BC:(c + 1) * BC]),
        }
        for c in range(N_CORES)
    ]
    r = run_bass_kernel_spmd(nc, in_maps, core_ids=list(range(N_CORES)))

    out = np.empty((B, 1), dtype=np.float32)
    for c in range(N_CORES):
        res = r.results[c]["res_out"]            # [128, NT]; row t*128+p = res[p, t]
        out[c * BC:(c + 1) * BC, 0] = res.T.reshape(BC) + np.float32(consts['e0'])
    return out


# Exposed for test harnesses that want the traced run.
def run_traced(**inputs):
    consts = _fold_constants(inputs)
    nc = _build_program(consts)
    z = np.ascontiguousarray(np.asarray(inputs['z'], dtype=np.float32))
    rw = np.ascontiguousarray(np.asarray(inputs['rw'], dtype=np.float32))
    in_maps = [
        {
            "z_in": np.ascontiguousarray(z[c * BC:(c + 1) * BC]),
            "rw_in": np.ascontiguousarray(rw[c * BC:(c + 1) * BC]),
        }
        for c in range(N_CORES)
    ]
    r = run_bass_kernel_spmd(nc, in_maps, core_ids=list(range(N_CORES)), trace=True)
    out = np.empty((B, 1), dtype=np.float32)
    for c in range(N_CORES):
        res = r.results[c]["res_out"]
        out[c * BC:(c + 1) * BC, 0] = res.T.reshape(BC) + np.float32(consts['e0'])
    return out, r


# revision 4
# speedup vs baseline: 1.1076x; 1.1076x over previous
# Trainium2 Bass kernel for nn_CovariantPotentialNet (B=4096, D=64, K=64, DM=512).
#
# The network collapses algebraically: tokens_x[b] = diag(rw[b]) @ chart_emb is
# rank-structured, so every DM=512-wide projection folds into small per-chart
# constants computed once on the host:
#   scores[b,k] = rw[b,k] * (z[b] @ A + a0)[k] / sqrt(DM) - geo * acosh(arg)^2
#   arg[b,k]    = 1 + 2*diff2[b,k] / ((1-|z[b]|^2) * (1-|c_k|^2))
#   out[b]      = sum_k softmax(scores)[b,k] * rw[b,k] * e[k] + e0
# with A [D,K], a0 [K], e [K], e0 scalar folded from the weight matrices
# (spectral norms included). The device kernel is pure data parallel over B:
# each of the 8 cores processes 512 rows (4 tiles of 128 on partitions).
#
# Per-core device program:
#   one DMA of packed [z|rw] [512,128]; one DMA of packed constants.
#   PE: 4 transposes (z tiles), then per tile 2 accumulating matmuls with
#   lhsT = [z.T] and [z.T^2] against folded weights; a rank-1 ones-row matmul
#   pre-adds the per-chart constants into PSUM.  The geo columns of PSUM then
#   hold diff2/cdiv and the S1 columns hold z@A + a0.
#   DVE/ACT: y = (diff2/cdiv)*(2/(1-zn)); arg = 1+y; d2 = ln(arg+sqrt(y(y+2)))^2;
#   scores = S1*rw/sqrt(DM) - geo*d2; p = exp(scores); out = sum(p*rw*e)/sum(p).
# A custom act-table json (sets: natural_log_exp / sqrt) keeps all ACT LUT
# loads except one off the critical path.
import json
import os
import sys
import tempfile

import numpy as np

for _p in ('/opt/trn_rl_repo', '/root/.axon_site/_ro/trn_rl_repo'):
    if _p not in sys.path:
        sys.path.append(_p)

import concourse.bass as bass
import concourse.mybir as mybir
import concourse.tile as tile
import concourse.bacc as bacc
from concourse.bass_utils import run_bass_kernel_spmd

F32 = mybir.dt.float32
N_CORES = 8
B, D, K, DM = 4096, 64, 64, 512
BC = B // N_CORES          # 512 rows per core
NT = BC // 128             # 4 tiles of 128 rows
ALU = mybir.AluOpType
ACTF = mybir.ActivationFunctionType
ACT_CFG_VERSION = 2        # bump when the act-table config changes (cache bust)

# Const block column layout ([128, CW] f32, single DMA)
_C_ID = 0            # identity [128, 0:128]
_C_GZ = 128          # gz [64, 128:256]
_C_GS = 256          # gs [64, 256:384]
_C_E = 384           # e broadcast [128, 384:448]
_C_CROW = 448        # crow4 [1, 448:960]
CW = 960


def _find_act_dir():
    import glob
    cands = glob.glob(
        '/nix/store/*/lib/python3*/site-packages/neuronxcc/pwp/pwp_bin_trainium')
    for c in cands:
        if os.path.exists(os.path.join(c, 'act_info.json')):
            return c
    return None


def _make_act_root():
    """Custom act_info.json limited to {natural_log_exp_and_others, sqrt_and_friends}
    so ln/exp share one LUT set; only one table switch reaches the critical
    path. Returns (json_path, tables) where tables matches the json's set
    order for bass's pre-placed LoadActFuncSet ids. (None, None) on surprise."""
    src_dir = _find_act_dir()
    if src_dir is None:
        return None, None
    try:
        info = json.load(open(os.path.join(src_dir, 'act_info.json')))
        keep = [s for s in info['act_func_sets']
                if s.get('name') in ('natural_log_exp_and_others', 'sqrt_and_friends')]
        if len(keep) != 2:
            return None, None
        # order: ln/exp set first so shared funcs resolve there
        keep.sort(key=lambda s: s['name'] != 'natural_log_exp_and_others')
        out_dir = tempfile.mkdtemp(prefix='act_root_')
        for s in keep:
            for k in info['pwp_file_keys']:
                fn = s[k]
                os.symlink(os.path.join(src_dir, fn), os.path.join(out_dir, fn))
        json.dump({'pwp_file_keys': info['pwp_file_keys'], 'act_func_sets': keep},
                  open(os.path.join(out_dir, 'act_info.json'), 'w'))
        tables = [
            (s['name'], {ACTF.from_pwp(v) for v in s['act'].keys()})
            for s in keep
        ]
        return os.path.join(out_dir, 'act_info.json'), tables
    except Exception:
        return None, None


class _Bacc(bacc.Bacc):
    """Bacc whose activation-table placement uses the filtered act_info
    (ids must index the json walrus sees via BASS_ACT_ROOT_JSON_PATH)."""

    _act_tables = None

    def insert_act_table_loads(self):
        if self._act_tables is None:
            return super().insert_act_table_loads()
        import bass_rust as _bass_rust
        has_activation = any(
            isinstance(i, mybir.InstActivation)
            for b in self.main_func.blocks
            for i in b.instructions
        )
        if not has_activation:
            return
        _bass_rust.insert_act_table_loads(self, list(self._act_tables))


def _fold_constants(inputs):
    """Host-side folding of all weights into small per-chart constants (float64)."""
    ii = {k: np.asarray(v).astype(np.float64) for k, v in inputs.items()}

    def l2n(x):
        return x / (np.linalg.norm(x) + 1e-12)

    def sscale(W, iters=5):
        u = l2n(np.ones(W.shape[0]))
        v = l2n(W.T @ u)
        for _ in range(iters):
            v = l2n(W.T @ u)
            u = l2n(W @ v)
        return W / (u @ (W @ v))

    Wz = sscale(ii['zW'])                     # [DM, D]
    vWs = sscale(ii['vW'])                    # [1, DM]
    cc = ii['chart_centers']
    n = np.linalg.norm(cc, axis=-1, keepdims=True)
    ccp = cc * np.minimum(1.0, (1.0 - 1e-5) / np.maximum(n, 1e-12))   # [K, D]
    cn = np.sum(ccp * ccp, axis=-1)           # [K]
    cdiv = 1.0 - cn                           # [K]

    Ek = ii['chart_emb'] @ ii['Wk'].T         # [K, DM]
    Ev = ii['chart_emb'] @ ii['Wv'].T         # [K, DM]
    A = Wz.T @ (ii['Wq'].T @ Ek.T)            # [D, K]
    a0 = (ii['zb'] @ ii['Wq'].T + ii['bq']) @ Ek.T     # [K]
    h = ii['Wo'].T @ vWs[0]                   # [DM]
    e = Ev @ h                                # [K]
    e0 = float(ii['bv'] @ h + ii['bo'] @ vWs[0] + ii['vb'][0])
    geo = float(ii['geo_scale'])

    cblock = np.zeros((128, CW), dtype=np.float32)
    cblock[:, _C_ID:_C_ID + 128] = np.eye(128, dtype=np.float32)
    cblock[0:D, _C_GZ + 0:_C_GZ + K] = A.astype(np.float32)
    cblock[0:D, _C_GZ + K:_C_GZ + 128] = (-2.0 * ccp / cdiv[:, None]).T.astype(np.float32)
    cblock[0:D, _C_GS + K:_C_GS + 128] = np.float32(1.0) / cdiv.astype(np.float32)[None, :]
    cblock[:, _C_E:_C_E + K] = e.astype(np.float32)[None, :]
    crow = np.zeros(128, dtype=np.float32)
    crow[0:K] = a0.astype(np.float32)
    crow[K:128] = (cn / cdiv).astype(np.float32)
    cblock[0, _C_CROW:_C_CROW + 512] = np.tile(crow, NT)

    return {
        'cblock': cblock,
        'geo': float(geo),
        'e0': e0,
        'inv_sqrt': float(np.float32(1.0 / np.sqrt(float(DM)))),
    }


def _build_program(consts, act_tables=None):
    _Bacc._act_tables = act_tables
    nc = _Bacc()
    zrw_in = nc.dram_tensor("zrw_in", [BC, 128], F32, kind="ExternalInput")
    res_out = nc.dram_tensor("res_out", [128, NT], F32, kind="ExternalOutput")
    cb_d = nc.inline_tensor(consts['cblock'], name="c_blk")
    nc.inline_tensor(np.array([ACT_CFG_VERSION], dtype=np.int32), name="c_cfg")

    geo = consts['geo']
    inv_sqrt = consts['inv_sqrt']

    with tile.TileContext(nc) as tc:
        with (
            tc.tile_pool(name="sb", bufs=1) as sb,
            tc.tile_pool(name="ps", bufs=1, space=bass.MemorySpace.PSUM) as ps,
            tc.tile_pool(name="pst", bufs=2, space=bass.MemorySpace.PSUM) as pst,
        ):
            # ACT table warmup: load the sqrt set while DMAs are in flight
            dummy = sb.tile([1, 1], F32)
            nc.vector.memset(dummy[:], 1.0)
            nc.scalar.activation(dummy[:], dummy[:], ACTF.Sqrt)

            ones = sb.tile([1, 128], F32)
            nc.vector.memset(ones[:], 1.0)

            cblk = sb.tile([128, CW], F32)
            nc.sync.dma_start(cblk[:], cb_d[:])
            zrw = sb.tile([128, NT, 128], F32)
            nc.sync.dma_start(zrw[:], zrw_in.rearrange("(t p) d -> p t d", p=128))
            z_v = zrw[:, :, 0:D]          # [128, NT, 64]
            rw_v = zrw[:, :, D:128]       # [128, NT, 64]
            ident = cblk[:, _C_ID:_C_ID + 128]
            gz = cblk[0:D, _C_GZ:_C_GZ + 128]
            gs = cblk[0:D, _C_GS:_C_GS + 128]
            e_bc = cblk[:, _C_E:_C_E + K]
            crow4 = cblk[0:1, _C_CROW:_C_CROW + 512]

            psum_g = ps.tile([128, NT, 128], F32)
            # rank-1 pre-add of per-chart constants into all four tiles
            nc.tensor.matmul(psum_g[:, :, :], ones[:], crow4,
                             start=True, stop=False, skip_group_check=True)

            zzt = sb.tile([D, NT, 128], F32)
            zsqt = sb.tile([D, NT, 128], F32)
            for t in range(NT):
                tp = pst.tile([D, 128], F32)
                nc.tensor.transpose(tp[:], z_v[:, t, :], ident)
                nc.vector.tensor_copy(zzt[:, t, :], tp[:])
                nc.vector.tensor_mul(zsqt[:, t, :], zzt[:, t, :], zzt[:, t, :])
                nc.tensor.matmul(psum_g[:, t, :], zzt[:, t, :], gz,
                                 start=False, stop=False, skip_group_check=True)
                nc.tensor.matmul(psum_g[:, t, :], zsqt[:, t, :], gs,
                                 start=False, stop=(t == NT - 1),
                                 skip_group_check=True)

            # zn = |z|^2 per row (DVE, overlaps the PE phase)
            zsqn = sb.tile([128, NT, D], F32)
            nc.vector.tensor_mul(zsqn[:], z_v, z_v)
            zn = sb.tile([128, NT], F32)
            nc.vector.reduce_sum(zn[:], zsqn[:], axis=mybir.AxisListType.X)
            zd = sb.tile([128, NT], F32)
            nc.vector.tensor_scalar(out=zd[:], in0=zn[:], scalar1=-0.5,
                                    scalar2=0.5, op0=ALU.mult, op1=ALU.add)
            izd = sb.tile([128, NT], F32)
            nc.vector.reciprocal(izd[:], zd[:])

            # rwe = rw * e (ready early; only needs the data DMA)
            rwe = sb.tile([128, NT, K], F32)
            e_b = e_bc.to_broadcast([128, K, NT]).rearrange("p k t -> p t k")
            nc.vector.tensor_tensor(out=rwe[:], in0=rw_v, in1=e_b, op=ALU.mult)

            # y = (diff2/cdiv) * (2/(1-zn));  arg = 1 + y  (clamped)
            y = sb.tile([128, NT, K], F32)
            izd_b = izd[:].to_broadcast([128, NT, K])
            nc.vector.tensor_tensor(out=y[:], in0=psum_g[:, :, K:128], in1=izd_b,
                                    op=ALU.mult)
            nc.vector.tensor_scalar_max(y[:], y[:], 1e-7)
            # d2 = ln(arg + sqrt(arg^2-1))^2, arg^2-1 = y*(y+2)
            v = sb.tile([128, NT, K], F32)
            nc.vector.scalar_tensor_tensor(out=v[:], in0=y[:], scalar=2.0,
                                           in1=y[:], op0=ALU.add, op1=ALU.mult)
            w = sb.tile([128, NT, K], F32)
            nc.scalar.activation(w[:], v[:], ACTF.Sqrt)
            t4 = sb.tile([128, NT, K], F32)
            nc.vector.scalar_tensor_tensor(out=t4[:], in0=y[:], scalar=1.0,
                                           in1=w[:], op0=ALU.add, op1=ALU.add)
            dl = sb.tile([128, NT, K], F32)
            nc.scalar.activation(dl[:], t4[:], ACTF.Ln)
            dsq = sb.tile([128, NT, K], F32)
            nc.vector.tensor_mul(dsq[:], dl[:], dl[:])

            # scores = (S1 * inv_sqrt) * rw - geo * d2
            sc = sb.tile([128, NT, K], F32)
            nc.vector.scalar_tensor_tensor(out=sc[:], in0=psum_g[:, :, 0:K],
                                           scalar=inv_sqrt, in1=rw_v,
                                           op0=ALU.mult, op1=ALU.mult)
            sco = sb.tile([128, NT, K], F32)
            nc.vector.scalar_tensor_tensor(out=sco[:], in0=dsq[:], scalar=-geo,
                                           in1=sc[:], op0=ALU.mult, op1=ALU.add)

            # softmax-weighted sum (scores in [-2.3,-0.4]: no max-shift needed)
            p = sb.tile([128, NT, K], F32)
            nc.scalar.activation(p[:], sco[:], ACTF.Exp)
            s = sb.tile([128, NT], F32)
            nc.vector.reduce_sum(s[:], p[:], axis=mybir.AxisListType.X)
            prw = sb.tile([128, NT, K], F32)
            nc.vector.tensor_mul(prw[:], p[:], rwe[:])
            num = sb.tile([128, NT], F32)
            nc.vector.reduce_sum(num[:], prw[:], axis=mybir.AxisListType.X)
            rs = sb.tile([128, NT], F32)
            nc.vector.reciprocal(rs[:], s[:])
            res = sb.tile([128, NT], F32)
            nc.vector.tensor_mul(res[:], num[:], rs[:])

            nc.sync.dma_start(res_out[:], res[:])

    nc.compile()
    return nc


def _run(inputs, trace=False):
    consts = _fold_constants(inputs)
    act_root, act_tables = _make_act_root()
    saved = os.environ.get('BASS_ACT_ROOT_JSON_PATH')
    try:
        if act_root is not None:
            os.environ['BASS_ACT_ROOT_JSON_PATH'] = act_root
        nc = _build_program(consts, act_tables)
        z = np.asarray(inputs['z'], dtype=np.float32)
        rw = np.asarray(inputs['rw'], dtype=np.float32)
        zrw = np.concatenate([z, rw], axis=1)          # [B, 128]
        in_maps = [{"zrw_in": np.ascontiguousarray(zrw[c * BC:(c + 1) * BC])}
                   for c in range(N_CORES)]
        r = run_bass_kernel_spmd(nc, in_maps, core_ids=list(range(N_CORES)),
                                 trace=trace)
    finally:
        if saved is None:
            os.environ.pop('BASS_ACT_ROOT_JSON_PATH', None)
        else:
            os.environ['BASS_ACT_ROOT_JSON_PATH'] = saved
    out = np.empty((B, 1), dtype=np.float32)
    for c in range(N_CORES):
        res = r.results[c]["res_out"]            # [128, NT]; row t*128+p = res[p, t]
        out[c * BC:(c + 1) * BC, 0] = res.T.reshape(BC) + np.float32(consts['e0'])
    return out, r


def kernel(**inputs):
    out, _ = _run(inputs, trace=False)
    return out


def run_traced(**inputs):
    return _run(inputs, trace=True)
